# revision 32
# baseline (speedup 1.0000x reference)
"""Trainium2 Bass kernel for nn_LogicConv3d (differentiable-logic conv tree).

Problem (hardcoded): x [16,64,32,32] f32; idx_a/idx_b [64,900,64,3] i32;
w0..w6 [s,64,16] f32 (s = 64,32,16,8,4,2,1). Output [16,64,900,1] f32.

Math: per (kernel k, window p): gather 64 (a,b) leaf pairs from x, blend each
pair with soft-gate coefficients (softmax(w) @ GATE_M), then 6 more pairwise
tree levels.  mix(a,b) = c0 + c1*a + c2*b + c3*a*b.

Mapping (v2, fp16):
 - Sharding: 8 cores = 4 batch-groups x 2 kernel-halves.  Core c handles
   batches [4g,4g+4) (g=c%4) and kernels [32h,32h+32) (h=c//4).  The device
   program is identical across cores; tables and the x-slice differ.
 - x-slice is stored fp16, b4-interleaved ([C,H,W,4] flat).  Each leaf
   (node,kern) lane gathers ONE contiguous 3840-elem run (30 rows x 32 cols
   x 4 batches) via indirect DMA -> [128, 3840] tile; cols w=30,31 are junk
   (row wrap) and simply never touched by the valid views.
 - All tree tensors fp16.  Mix is computed as
       u = c3*a + c2   (ScalarE ACT, per-lane scale/bias)
       w = c1*a + c0   (DVE tensor_scalar 4x fp16, or ScalarE for balance)
       q = u * b       (DVE tensor_tensor, 2x fp16, in-place over u)
       r = q + w       (DVE tensor_tensor, 2x fp16)
   which is exact (= c0 + c1*a + c2*b + c3*ab) with NO constant folding and
   avoids scalar_tensor_tensor entirely (stt has no 2x uop -> 1 elem/cycle).
 - Partition layout: lane = 32*u + kern_local, node = key + (16>>level)*u.
   L5/L6 need partition realignment (equal-base operand constraint):
   done with small SBUF->SBUF DMAs.
"""
import numpy as np

B, C, H, W = 16, 64, 32, 32
K = 64
RF = 3
DEPTH = 6
S = 64
PW = 30            # windows per axis
P = PW * PW        # 900
NCORES = 8
B4 = 4             # batches per core
KH = 32            # kernels per core
FRUN = 30 * 32 * B4          # 3840: gather run / tile free size
XPAD = C * H * W * B4 + 4096  # fp16 elements in the padded x slice

GATE_M = np.array([
    [0, 0, 0, 0], [0, 0, 0, 1], [0, 1, 0, -1], [0, 1, 0, 0],
    [0, 0, 1, -1], [0, 0, 1, 0], [0, 1, 1, -2], [0, 1, 1, -1],
    [1, -1, -1, 1], [1, -1, -1, 2], [1, 0, -1, 0], [1, 0, -1, 1],
    [1, -1, 0, 0], [1, -1, 0, 1], [1, 0, 0, -1], [1, 0, 0, 0],
], dtype=np.float32)  # [16 gates, 4] -> c0,c1,c2,c3 = GATE_M.T @ softmax(w)

W_SCE_EVERY = 3    # every 3rd mix puts its w-op on ScalarE (engine balance)
ACCUM_L0 = False   # CCE-accum r-offload: measured net-negative (the 3x
                   # SBUF-port traffic of read-modify-write DMAs makes the
                   # leaf phase DMA-bound; DVE savings don't pay for it)


# ---------------------------------------------------------------------------
# static schedule
# ---------------------------------------------------------------------------
def _build_schedule():
    """Software-pipelined order over the merge tree.  Each entry:
    dict(level, key, lanes, node[lanes], hsplit) with lane = 32*u + kern,
    node = key + (16>>l)*u for l<=4; L5: two 32-lane ops (node=key);
    L6: 32 lanes node=0.  Merges are delayed by one leaf-pair so that
    producers finish well before consumers (hides DMA-accum latency).
    Ops after the last leaf (the drain chain) are marked hsplit for
    row-chunked tail pipelining."""
    from collections import deque

    def mk(l, key):
        if l <= 4:
            lanes = np.arange(128)
            node = key + (16 >> l) * (lanes >> 5)
        elif l == 5:
            node = np.arange(64) >> 5
        else:
            node = np.zeros(32, np.int64)
        return dict(level=l, key=key, lanes=len(node), node=node,
                    hsplit=False)

    children = {}
    for l in range(1, 5):
        for k in range(16 >> l):
            children[(l, k)] = [(l - 1, 2 * k), (l - 1, 2 * k + 1)]
    children[(5, 0)] = [(4, 0)]
    children[(6, 0)] = [(5, 0)]
    parents = {}
    for p, cs in children.items():
        for c in cs:
            parents.setdefault(c, []).append(p)

    order = []
    emitted = set()
    ready = deque()

    def emit(node):
        order.append(node)
        emitted.add(node)
        for p in parents.get(node, []):
            if p not in emitted and p not in ready and \
                    all(c in emitted for c in children[p]):
                ready.append(p)

    for pair in range(8):
        snap = len(ready)
        emit((0, 2 * pair))
        emit((0, 2 * pair + 1))
        for _ in range(snap):
            emit(ready.popleft())
    n_before_drain = len(order)
    while ready:
        emit(ready.popleft())

    ops = [mk(l, k) for (l, k) in order]
    for i in range(n_before_drain, len(ops)):
        ops[i]['drain'] = True
    for op in ops:
        op.setdefault('drain', False)
    ops[0]['hsplit'] = True   # fast head: overlap first gathers w/ compute
    ops[1]['hsplit'] = True
    ops[-1]['hsplit'] = True  # L6 in halves so the out-DMA overlaps r6
    return ops


_SCHED = _build_schedule()
_NMIX = len(_SCHED)          # 33
_NCOLS = 4 * _NMIX


def _softmax_f32(w):
    w = w.astype(np.float64)
    m = w.max(-1, keepdims=True)
    e = np.exp(w - m)
    return e / e.sum(-1, keepdims=True)


def _coef_tables(ws, khalf):
    """ws = [w0..w6]; khalf = kernel half (0/1).  Returns [128, _NCOLS] f32:
    cols 4i..4i+3 = (c3, c2, c1, c0) of mix i for this lane's (node, kern)."""
    cs = []
    for wl in ws:
        p = _softmax_f32(wl)                      # [s, K, 16] f64
        cs.append(np.einsum('skg,gj->skj', p, GATE_M.astype(np.float64)))
    coef = np.zeros((128, _NCOLS), dtype=np.float64)
    for i, op in enumerate(_SCHED):
        l, node, n = op['level'], op['node'], op['lanes']
        kern = KH * khalf + (np.arange(n) & 31)
        c = cs[l][node, kern]                     # [n, 4] = (c0,c1,c2,c3)
        coef[np.arange(n), 4 * i + 0] = c[:, 3]
        coef[np.arange(n), 4 * i + 1] = c[:, 2]
        coef[np.arange(n), 4 * i + 2] = c[:, 1]
        coef[np.arange(n), 4 * i + 3] = c[:, 0]
    return coef.astype(np.float32)


def _offset_tables(idx_a, idx_b, khalf):
    """Gather index tables [128, 32] i32: col = 2*t + side.
    Element offsets into the b4-interleaved fp16 x-slice."""
    offs = np.zeros((128, 32), dtype=np.int64)
    for op in _SCHED:
        if op['level'] != 0:
            continue
        t = op['key']
        kern = KH * khalf + (np.arange(128) & 31)
        for side, idx in ((0, idx_a), (1, idx_b)):
            ha = idx[kern, 0, op['node'], 0].astype(np.int64)
            wa = idx[kern, 0, op['node'], 1].astype(np.int64)
            ca = idx[kern, 0, op['node'], 2].astype(np.int64)
            offs[:, 2 * t + side] = (ca * (H * W) + ha * W + wa) * B4
    return offs.astype(np.int32)


# ---------------------------------------------------------------------------
# numpy emulator (mirrors the device schedule exactly; for validation)
# ---------------------------------------------------------------------------
def _emulate_core(xp, offs, coef):
    """xp: [XPAD] f16 slice. Returns [32, FRUN] f16 (junk cols included)."""
    f16, f32 = np.float16, np.float32
    tiles = {}

    def mix(i, a, b, n):
        c3 = coef[np.arange(n), 4 * i + 0][:, None].astype(f32)
        c2 = coef[np.arange(n), 4 * i + 1][:, None].astype(f32)
        c1 = coef[np.arange(n), 4 * i + 2][:, None].astype(f32)
        c0 = coef[np.arange(n), 4 * i + 3][:, None].astype(f32)
        u = (a.astype(f32) * c3 + c2).astype(f16)
        w = (a.astype(f32) * c1 + c0).astype(f16)
        q = (u.astype(f32) * b.astype(f32)).astype(f16)
        return (q.astype(f32) + w.astype(f32)).astype(f16)

    for i, op in enumerate(_SCHED):
        l, key, n = op['level'], op['key'], op['lanes']
        if l == 0:
            o = offs[:, 2 * key + 0]
            a = xp[o[:, None] + np.arange(FRUN)[None, :]]
            o = offs[:, 2 * key + 1]
            b = xp[o[:, None] + np.arange(FRUN)[None, :]]
        elif l < 5:
            a = tiles[(l - 1, 2 * key)]
            b = tiles[(l - 1, 2 * key + 1)]
        elif l == 5:
            t4 = tiles[(4, 0)]
            a = np.concatenate([t4[0:32], t4[64:96]])     # nodes 0,2
            b = np.concatenate([t4[32:64], t4[96:128]])   # nodes 1,3
        else:
            t5 = tiles[(5, 0)]
            a, b = t5[0:32], t5[32:64]
        tiles[(l, key)] = mix(i, a, b, n)
    return tiles[(6, 0)]


# ---------------------------------------------------------------------------
# Bass program (built once, cached)
# ---------------------------------------------------------------------------
_BASS_CACHE = {}


def _build_bass():
    if 'nc' in _BASS_CACHE:
        return _BASS_CACHE['nc']
    import concourse.bass as bass
    import concourse.mybir as mybir
    import concourse.tile as tile
    import concourse.bacc as bacc

    f16 = mybir.dt.float16
    f32 = mybir.dt.float32
    nc = bacc.Bacc("TRN2", target_bir_lowering=False, debug=False,
                   num_devices=NCORES)
    xsrc_d = nc.dram_tensor("xsrc", [XPAD, 1], f16, kind="ExternalInput").ap()
    offs_d = nc.dram_tensor("offs", [128, 32], mybir.dt.int32,
                            kind="ExternalInput").ap()
    coef_d = nc.dram_tensor("coef", [128, _NCOLS], f32,
                            kind="ExternalInput").ap()
    out_d = nc.dram_tensor("out", [32, FRUN], f16, kind="ExternalOutput").ap()

    AL = mybir.AluOpType
    ACTF = mybir.ActivationFunctionType

    HF = FRUN // 2     # 1920: h-half size

    def vvr(ap, r0, r1):
        # valid view of rows [r0,r1) of a [n, FRUN] AP (skip w=30,31 junk)
        return ap.rearrange("p (h wb) -> p h wb", h=30, wb=128)[
            :, r0:r1, 0:120]

    def r3d(ap):
        # [n, FRUN] -> [n, 2, 1920] (CCE descriptor length <= 2048 elems)
        return ap.rearrange("p (a b) -> p a b", a=2, b=HF)

    HALVES = [(0, 15), (15, 30)]
    QUARTERS = [(0, 8), (8, 15), (15, 23), (23, 30)]
    FULL = [(0, 30)]

    with tile.TileContext(nc) as tc:
        with (
            tc.tile_pool(name="const", bufs=1) as pc,
            tc.tile_pool(name="ab", bufs=3) as pab,
            tc.tile_pool(name="t0p", bufs=4) as pt0,
            tc.tile_pool(name="lv1", bufs=3) as plv1,
            tc.tile_pool(name="lv2", bufs=2) as plv2,
            tc.tile_pool(name="t4p", bufs=1) as pt4,
            tc.tile_pool(name="tmp", bufs=2) as ptmp,
            tc.tile_pool(name="fin", bufs=1) as pfin,
        ):
            offs_t = pc.tile([128, 32], mybir.dt.int32, tag="offs",
                             name="offs_t")
            nc.gpsimd.dma_start(offs_t[:], offs_d[:])
            coef_t = pc.tile([128, _NCOLS], f32, tag="coef", name="coef_t")
            nc.sync.dma_start(coef_t[:], coef_d[:])
            warm_t = pc.tile([1, 8], f32, tag="warm", name="warm_t")
            nc.scalar.activation(warm_t[:], coef_t[0:1, 0:8],
                                 ACTF.Identity, bias=0.0, scale=1.0)

            def gather(t, split=False):
                at = pab.tile([128, FRUN], f16, tag="A", name=f"a{t}")
                bt = pab.tile([128, FRUN], f16, tag="B", name=f"b{t}")
                for side, dst in ((0, at), (1, bt)):
                    ioff = bass.IndirectOffsetOnAxis(
                        ap=offs_t[:, 2 * t + side:2 * t + side + 1], axis=0)
                    if split:
                        for (r0, r1) in QUARTERS:
                            nc.gpsimd.indirect_dma_start(
                                out=dst[:, 128 * r0:128 * r1],
                                out_offset=None, in_=xsrc_d[:],
                                in_offset=ioff, element_offset=128 * r0)
                    else:
                        nc.gpsimd.indirect_dma_start(
                            out=dst[:], out_offset=None, in_=xsrc_d[:],
                            in_offset=ioff)
                return at, bt

            gtiles = {0: gather(0, True), 1: gather(1, True)}
            tiles = {}

            def resolve_ab(i):
                """(a_ap, b_ap, base) for mix i; base = partition offset
                the u/w/q tmp lanes must live at (to match b for TT)."""
                op = _SCHED[i]
                l, key = op['level'], op['key']
                if l == 0:
                    at, bt = gtiles[key]
                    return at[:], bt[:], 0
                if l < 5:
                    return (tiles[(l - 1, 2 * key)][:],
                            tiles[(l - 1, 2 * key + 1)][:], 0)
                if l == 5:
                    a5, b5 = tiles['A5'], tiles['B5']
                    return a5[:], b5[:], 0
                t5 = tiles[(5, 0)]
                return t5[0:32, :], t5[32:64, :], 32

            def coefs(i, n):
                return [coef_t[0:n, 4 * i + j:4 * i + j + 1]
                        for j in range(4)]

            def w_on_sce(i):
                op = _SCHED[i]
                return (op['drain'] and op['level'] == 4) or \
                    (not op['drain'] and not op['hsplit']
                     and i % W_SCE_EVERY == W_SCE_EVERY - 1)

            def emit_u(i):
                """Pre-emit mix i's u-op (and its w-op when it rides ScalarE)
                — the lookahead keeps ScE a mix ahead of DVE."""
                op = _SCHED[i]
                n = op['lanes']
                a_ap, _, base = resolve_ab(i)
                c3, c2, c1, c0 = coefs(i, n)
                u_t = ptmp.tile([128, FRUN], f16, tag="u", name=f"u{i}")
                u_ap = u_t[base:base + n, :]
                w_ahead = None
                if w_on_sce(i):
                    w_t = ptmp.tile([128, FRUN], f16, tag="w", name=f"w{i}")
                    w_ahead = w_t[base:base + n, :]
                chunks = (QUARTERS if i <= 1 else HALVES) \
                    if op['hsplit'] else FULL
                for (r0, r1) in chunks:
                    uv, av = vvr(u_ap, r0, r1), vvr(a_ap, r0, r1)
                    if op['drain'] and op['level'] >= 4:
                        nc.vector.tensor_scalar(uv, av, c3, c2,
                                                AL.mult, AL.add)
                    else:
                        nc.scalar.activation(uv, av, ACTF.Identity,
                                             bias=c2, scale=c3)
                    if w_ahead is not None:
                        nc.scalar.activation(vvr(w_ahead, r0, r1), av,
                                             ACTF.Identity, bias=c0, scale=c1)
                return u_t, u_ap, w_ahead

            def can_lookahead(i):
                if i + 1 >= _NMIX:
                    return False
                nxt = _SCHED[i + 1]
                if nxt['level'] == 5:
                    return False   # A5/B5 not realigned yet
                if nxt['level'] == 0:
                    return True
                a_child = ((5, 0) if nxt['level'] == 6 else
                           (nxt['level'] - 1, 2 * nxt['key']))
                cur = _SCHED[i]
                return a_child != (cur['level'], cur['key'])

            pend_accum = []
            pending_u = None
            for i, op in enumerate(_SCHED):
                l, key, n = op['level'], op['key'], op['lanes']
                _, _, c1, c0 = coefs(i, n)

                if l == 0 and key + 2 < 16:
                    gtiles[key + 2] = gather(key + 2)
                if pend_accum:
                    rp, up = pend_accum.pop(0)
                    nc.gpsimd.dma_start(out=r3d(rp[:]), in_=r3d(up[:]),
                                        accum_op=AL.add)
                if l == 5:
                    # realign nodes {0,2} / {1,3} of T4 into A5 / B5;
                    # A5 first (gates u5), spread over both HWDGE queues
                    t4 = tiles[(4, 0)]
                    a5 = pfin.tile([64, FRUN], f16, tag="A5", name="a5")
                    b5 = pfin.tile([64, FRUN], f16, tag="B5", name="b5")
                    nc.sync.dma_start(a5[0:32, :], t4[0:32, :])
                    nc.scalar.dma_start(a5[32:64, :], t4[64:96, :])
                    nc.sync.dma_start(b5[0:32, :], t4[32:64, :])
                    nc.scalar.dma_start(b5[32:64, :], t4[96:128, :])
                    tiles['A5'] = a5
                    tiles['B5'] = b5

                a_ap, b_ap, base = resolve_ab(i)
                if pending_u is not None and pending_u[0] == i:
                    u_t, u_ap, w_ahead = pending_u[1]
                else:
                    u_t, u_ap, w_ahead = emit_u(i)
                pending_u = None

                # output tile
                if l == 0:
                    r_t = pt0.tile([128, FRUN], f16, tag="T0",
                                   name=f"t0_{key}")
                elif l == 1:
                    r_t = plv1.tile([128, FRUN], f16, tag="T1",
                                    name=f"t1_{key}")
                elif l < 4:
                    r_t = plv2.tile([128, FRUN], f16, tag=f"T{l}",
                                    name=f"t{l}_{key}")
                elif l == 4:
                    r_t = pt4.tile([128, FRUN], f16, tag="T4", name="t4")
                else:
                    r_t = pfin.tile([n, FRUN], f16, tag=f"T{l}",
                                    name=f"t{l}")
                tiles[(l, key)] = r_t

                accum = (ACCUM_L0 and l == 0 and key % 2 == 0
                         and not op['hsplit'])
                if accum:
                    w_ap = r_t[:]
                elif w_ahead is not None:
                    w_ap = w_ahead
                else:
                    w_t = ptmp.tile([128, FRUN], f16, tag="w", name=f"w{i}")
                    w_ap = w_t[base:base + n, :]

                chunks = (QUARTERS if i <= 1 else HALVES) \
                    if op['hsplit'] else FULL
                for (r0, r1) in chunks:
                    av, bv = vvr(a_ap, r0, r1), vvr(b_ap, r0, r1)
                    uv, wv = vvr(u_ap, r0, r1), vvr(w_ap, r0, r1)
                    rv = vvr(r_t[:], r0, r1)
                    if w_ahead is None:
                        nc.vector.tensor_scalar(wv, av, c1, c0,
                                                AL.mult, AL.add)
                    nc.vector.tensor_tensor(uv, uv, bv, AL.mult)
                    if (r0, r1) == chunks[-1] and can_lookahead(i):
                        pending_u = (i + 1, emit_u(i + 1))
                    if not accum:
                        nc.vector.tensor_tensor(rv, uv, wv, AL.add)
                        if l == 6:
                            sl = slice(128 * r0, 128 * r1)
                            nc.sync.dma_start(out_d[:, sl], r_t[:, sl])
                if accum:
                    pend_accum.append((r_t, u_t))
    nc.compile()
    _BASS_CACHE['nc'] = nc
    return nc


def _prep_inputs(x, idx_a, idx_b, ws):
    x = np.ascontiguousarray(x, dtype=np.float32)
    in_maps = []
    for core in range(NCORES):
        g, h = core % 4, core // 4
        coef = _coef_tables(ws, h)
        offs = _offset_tables(idx_a, idx_b, h)
        # b4-interleaved slice: [C,H,W,B4] fp16
        xs = x[B4 * g:B4 * g + B4].transpose(1, 2, 3, 0)
        xp = np.zeros((XPAD,), dtype=np.float16)
        xp[:B4 * C * H * W] = xs.reshape(-1).astype(np.float16)
        in_maps.append({"xsrc": xp.reshape(XPAD, 1), "offs": offs,
                        "coef": coef})
    return in_maps


def _assemble(core_outs):
    """core_outs: list of [32, FRUN] f16 -> [16,64,900,1] f32."""
    full = np.empty((B, K, P, 1), dtype=np.float32)
    for core, o in enumerate(core_outs):
        g, h = core % 4, core // 4
        v = np.asarray(o).reshape(KH, PW, 32, B4)[:, :, 0:PW, :]  # k,hh,ww,b
        v = v.astype(np.float32).transpose(3, 0, 1, 2)            # b,k,hh,ww
        full[B4 * g:B4 * g + B4, KH * h:KH * h + KH] = \
            v.reshape(B4, KH, P, 1)
    return np.ascontiguousarray(full)


def kernel(x, idx_a, idx_b, w0, w1, w2, w3, w4, w5, w6):
    ws = [np.asarray(w, dtype=np.float32) for w in
          (w0, w1, w2, w3, w4, w5, w6)]
    x = np.asarray(x, dtype=np.float32)
    idx_a = np.asarray(idx_a, dtype=np.int32)
    idx_b = np.asarray(idx_b, dtype=np.int32)
    in_maps = _prep_inputs(x, idx_a, idx_b, ws)
    nc = _build_bass()
    from concourse.bass_utils import run_bass_kernel_spmd
    res = run_bass_kernel_spmd(nc, in_maps, core_ids=list(range(NCORES)))
    return _assemble([r["out"] for r in res.results])


def kernel_emulate(x, idx_a, idx_b, w0, w1, w2, w3, w4, w5, w6):
    """Pure-numpy emulation of the exact device schedule (debug aid)."""
    ws = [np.asarray(w, dtype=np.float32) for w in
          (w0, w1, w2, w3, w4, w5, w6)]
    in_maps = _prep_inputs(np.asarray(x, np.float32),
                           np.asarray(idx_a, np.int32),
                           np.asarray(idx_b, np.int32), ws)
    outs = [_emulate_core(m["xsrc"].reshape(-1), m["offs"], m["coef"])
            for m in in_maps]
    return _assemble(outs)


# revision 37
# speedup vs baseline: 1.0650x; 1.0650x over previous
"""Trainium2 Bass kernel for nn_LogicConv3d (differentiable-logic conv tree).

Problem (hardcoded): x [16,64,32,32] f32; idx_a/idx_b [64,900,64,3] i32;
w0..w6 [s,64,16] f32 (s = 64,32,16,8,4,2,1). Output [16,64,900,1] f32.

Math: per (kernel k, window p): gather 64 (a,b) leaf pairs from x, blend each
pair with soft-gate coefficients (softmax(w) @ GATE_M), then 6 more pairwise
tree levels.  mix(a,b) = c0 + c1*a + c2*b + c3*a*b.

Mapping (v2, fp16):
 - Sharding: 8 cores = 4 batch-groups x 2 kernel-halves.  Core c handles
   batches [4g,4g+4) (g=c%4) and kernels [32h,32h+32) (h=c//4).  The device
   program is identical across cores; tables and the x-slice differ.
 - x-slice is stored fp16, b4-interleaved ([C,H,W,4] flat).  Each leaf
   (node,kern) lane gathers ONE contiguous 3840-elem run (30 rows x 32 cols
   x 4 batches) via indirect DMA -> [128, 3840] tile; cols w=30,31 are junk
   (row wrap) and simply never touched by the valid views.
 - All tree tensors fp16.  Mix is computed as
       u = c3*a + c2   (ScalarE ACT, per-lane scale/bias)
       w = c1*a + c0   (DVE tensor_scalar 4x fp16, or ScalarE for balance)
       q = u * b       (DVE tensor_tensor, 2x fp16, in-place over u)
       r = q + w       (DVE tensor_tensor, 2x fp16)
   which is exact (= c0 + c1*a + c2*b + c3*ab) with NO constant folding and
   avoids scalar_tensor_tensor entirely (stt has no 2x uop -> 1 elem/cycle).
 - Partition layout: lane = 32*u + kern_local, node = key + (16>>level)*u.
   L5/L6 need partition realignment (equal-base operand constraint):
   done with small SBUF->SBUF DMAs.
"""
import numpy as np

B, C, H, W = 16, 64, 32, 32
K = 64
RF = 3
DEPTH = 6
S = 64
PW = 30            # windows per axis
P = PW * PW        # 900
NCORES = 8
B4 = 4             # batches per core
KH = 32            # kernels per core
FRUN = 30 * 32 * B4          # 3840: gather run / tile free size
XPAD = C * H * W * B4 + 4096  # fp16 elements in the padded x slice

GATE_M = np.array([
    [0, 0, 0, 0], [0, 0, 0, 1], [0, 1, 0, -1], [0, 1, 0, 0],
    [0, 0, 1, -1], [0, 0, 1, 0], [0, 1, 1, -2], [0, 1, 1, -1],
    [1, -1, -1, 1], [1, -1, -1, 2], [1, 0, -1, 0], [1, 0, -1, 1],
    [1, -1, 0, 0], [1, -1, 0, 1], [1, 0, 0, -1], [1, 0, 0, 0],
], dtype=np.float32)  # [16 gates, 4] -> c0,c1,c2,c3 = GATE_M.T @ softmax(w)

W_SCE_EVERY = 3    # every 3rd mix puts its w-op on ScalarE (engine balance)
ACCUM_L0 = False   # CCE-accum r-offload: measured net-negative (the 3x
                   # SBUF-port traffic of read-modify-write DMAs makes the
                   # leaf phase DMA-bound; DVE savings don't pay for it)


# ---------------------------------------------------------------------------
# static schedule
# ---------------------------------------------------------------------------
def _build_schedule():
    """Software-pipelined order over the merge tree.  Each entry:
    dict(level, key, lanes, node[lanes], hsplit) with lane = 32*u + kern,
    node = key + (16>>l)*u for l<=4; L5: two 32-lane ops (node=key);
    L6: 32 lanes node=0.  Merges are delayed by one leaf-pair so that
    producers finish well before consumers (hides DMA-accum latency).
    Ops after the last leaf (the drain chain) are marked hsplit for
    row-chunked tail pipelining."""
    from collections import deque

    def mk(l, key):
        if l <= 4:
            lanes = np.arange(128)
            node = key + (16 >> l) * (lanes >> 5)
        elif l == 5:
            node = np.arange(64) >> 5
        else:
            node = np.zeros(32, np.int64)
        return dict(level=l, key=key, lanes=len(node), node=node,
                    hsplit=False)

    children = {}
    for l in range(1, 5):
        for k in range(16 >> l):
            children[(l, k)] = [(l - 1, 2 * k), (l - 1, 2 * k + 1)]
    children[(5, 0)] = [(4, 0)]
    children[(6, 0)] = [(5, 0)]
    parents = {}
    for p, cs in children.items():
        for c in cs:
            parents.setdefault(c, []).append(p)

    order = []
    emitted = set()
    ready = deque()

    def emit(node):
        order.append(node)
        emitted.add(node)
        for p in parents.get(node, []):
            if p not in emitted and p not in ready and \
                    all(c in emitted for c in children[p]):
                ready.append(p)

    for pair in range(8):
        snap = len(ready)
        emit((0, 2 * pair))
        emit((0, 2 * pair + 1))
        for _ in range(snap):
            emit(ready.popleft())
    n_before_drain = len(order)
    while ready:
        emit(ready.popleft())

    ops = [mk(l, k) for (l, k) in order]
    for i in range(n_before_drain, len(ops)):
        ops[i]['drain'] = True
    for op in ops:
        op.setdefault('drain', False)
    ops[0]['hsplit'] = True   # fast head: overlap first gathers w/ compute
    ops[1]['hsplit'] = True
    ops[-3]['hsplit'] = True  # L4 in halves so the L5 realigns overlap r4
    return ops


_SCHED = _build_schedule()
_NMIX = len(_SCHED)          # 33
_NCOLS = 4 * _NMIX


def _softmax_f32(w):
    w = w.astype(np.float64)
    m = w.max(-1, keepdims=True)
    e = np.exp(w - m)
    return e / e.sum(-1, keepdims=True)


def _coef_tables(ws, khalf):
    """ws = [w0..w6]; khalf = kernel half (0/1).  Returns [128, _NCOLS] f32:
    cols 4i..4i+3 = (c3, c2, c1, c0) of mix i for this lane's (node, kern)."""
    cs = []
    for wl in ws:
        p = _softmax_f32(wl)                      # [s, K, 16] f64
        cs.append(np.einsum('skg,gj->skj', p, GATE_M.astype(np.float64)))
    coef = np.zeros((128, _NCOLS), dtype=np.float64)
    for i, op in enumerate(_SCHED):
        l, node, n = op['level'], op['node'], op['lanes']
        kern = KH * khalf + (np.arange(n) & 31)
        c = cs[l][node, kern]                     # [n, 4] = (c0,c1,c2,c3)
        coef[np.arange(n), 4 * i + 0] = c[:, 3]
        coef[np.arange(n), 4 * i + 1] = c[:, 2]
        coef[np.arange(n), 4 * i + 2] = c[:, 1]
        coef[np.arange(n), 4 * i + 3] = c[:, 0]
    return coef.astype(np.float32)


def _offset_tables(idx_a, idx_b, khalf):
    """Gather index tables [128, 32] i32: col = 2*t + side.
    Element offsets into the b4-interleaved fp16 x-slice."""
    offs = np.zeros((128, 32), dtype=np.int64)
    for op in _SCHED:
        if op['level'] != 0:
            continue
        t = op['key']
        kern = KH * khalf + (np.arange(128) & 31)
        for side, idx in ((0, idx_a), (1, idx_b)):
            ha = idx[kern, 0, op['node'], 0].astype(np.int64)
            wa = idx[kern, 0, op['node'], 1].astype(np.int64)
            ca = idx[kern, 0, op['node'], 2].astype(np.int64)
            offs[:, 2 * t + side] = (ca * (H * W) + ha * W + wa) * B4
    return offs.astype(np.int32)


# ---------------------------------------------------------------------------
# numpy emulator (mirrors the device schedule exactly; for validation)
# ---------------------------------------------------------------------------
def _emulate_core(xp, offs, coef):
    """xp: [XPAD] f16 slice. Returns [32, FRUN] f16 (junk cols included)."""
    f16, f32 = np.float16, np.float32
    tiles = {}

    def mix(i, a, b, n):
        c3 = coef[np.arange(n), 4 * i + 0][:, None].astype(f32)
        c2 = coef[np.arange(n), 4 * i + 1][:, None].astype(f32)
        c1 = coef[np.arange(n), 4 * i + 2][:, None].astype(f32)
        c0 = coef[np.arange(n), 4 * i + 3][:, None].astype(f32)
        u = (a.astype(f32) * c3 + c2).astype(f16)
        w = (a.astype(f32) * c1 + c0).astype(f16)
        q = (u.astype(f32) * b.astype(f32)).astype(f16)
        return (q.astype(f32) + w.astype(f32)).astype(f16)

    for i, op in enumerate(_SCHED):
        l, key, n = op['level'], op['key'], op['lanes']
        if l == 0:
            o = offs[:, 2 * key + 0]
            a = xp[o[:, None] + np.arange(FRUN)[None, :]]
            o = offs[:, 2 * key + 1]
            b = xp[o[:, None] + np.arange(FRUN)[None, :]]
        elif l < 5:
            a = tiles[(l - 1, 2 * key)]
            b = tiles[(l - 1, 2 * key + 1)]
        elif l == 5:
            t4 = tiles[(4, 0)]
            a = np.concatenate([t4[0:32], t4[64:96]])     # nodes 0,2
            b = np.concatenate([t4[32:64], t4[96:128]])   # nodes 1,3
        else:
            t5 = tiles[(5, 0)]
            a, b = t5[0:32], t5[32:64]
        tiles[(l, key)] = mix(i, a, b, n)
    return tiles[(6, 0)]


# ---------------------------------------------------------------------------
# Bass program (built once, cached)
# ---------------------------------------------------------------------------
_BASS_CACHE = {}


def _build_bass():
    if 'nc' in _BASS_CACHE:
        return _BASS_CACHE['nc']
    import concourse.bass as bass
    import concourse.mybir as mybir
    import concourse.tile as tile
    import concourse.bacc as bacc

    f16 = mybir.dt.float16
    f32 = mybir.dt.float32
    nc = bacc.Bacc("TRN2", target_bir_lowering=False, debug=False,
                   num_devices=NCORES)
    xsrc_d = nc.dram_tensor("xsrc", [XPAD, 1], f16, kind="ExternalInput").ap()
    offs_d = nc.dram_tensor("offs", [128, 32], mybir.dt.int32,
                            kind="ExternalInput").ap()
    coef_d = nc.dram_tensor("coef", [128, _NCOLS], f32,
                            kind="ExternalInput").ap()
    out_d = nc.dram_tensor("out", [32, FRUN], f16, kind="ExternalOutput").ap()

    AL = mybir.AluOpType
    ACTF = mybir.ActivationFunctionType

    HF = FRUN // 2     # 1920: h-half size

    def vvr(ap, r0, r1):
        # valid view of rows [r0,r1) of a [n, FRUN] AP (skip w=30,31 junk)
        return ap.rearrange("p (h wb) -> p h wb", h=30, wb=128)[
            :, r0:r1, 0:120]

    def r3d(ap):
        # [n, FRUN] -> [n, 2, 1920] (CCE descriptor length <= 2048 elems)
        return ap.rearrange("p (a b) -> p a b", a=2, b=HF)

    HALVES = [(0, 15), (15, 30)]
    QUARTERS = [(0, 8), (8, 15), (15, 23), (23, 30)]
    FULL = [(0, 30)]

    with tile.TileContext(nc) as tc:
        with (
            tc.tile_pool(name="const", bufs=1) as pc,
            tc.tile_pool(name="ab", bufs=3) as pab,
            tc.tile_pool(name="t0p", bufs=4) as pt0,
            tc.tile_pool(name="lv1", bufs=3) as plv1,
            tc.tile_pool(name="lv2", bufs=2) as plv2,
            tc.tile_pool(name="t4p", bufs=1) as pt4,
            tc.tile_pool(name="tmp", bufs=2) as ptmp,
            tc.tile_pool(name="fin", bufs=1) as pfin,
        ):
            offs_t = pc.tile([128, 32], mybir.dt.int32, tag="offs",
                             name="offs_t")
            nc.gpsimd.dma_start(offs_t[:], offs_d[:])
            coef_t = pc.tile([128, _NCOLS], f32, tag="coef", name="coef_t")
            nc.sync.dma_start(coef_t[:], coef_d[:])
            warm_t = pc.tile([1, 8], f32, tag="warm", name="warm_t")
            nc.scalar.activation(warm_t[:], coef_t[0:1, 0:8],
                                 ACTF.Identity, bias=0.0, scale=1.0)

            def gather(t, split=False):
                at = pab.tile([128, FRUN], f16, tag="A", name=f"a{t}")
                bt = pab.tile([128, FRUN], f16, tag="B", name=f"b{t}")
                for side, dst in ((0, at), (1, bt)):
                    ioff = bass.IndirectOffsetOnAxis(
                        ap=offs_t[:, 2 * t + side:2 * t + side + 1], axis=0)
                    if split:
                        for (r0, r1) in HALVES:
                            nc.gpsimd.indirect_dma_start(
                                out=dst[:, 128 * r0:128 * r1],
                                out_offset=None, in_=xsrc_d[:],
                                in_offset=ioff, element_offset=128 * r0)
                    else:
                        nc.gpsimd.indirect_dma_start(
                            out=dst[:], out_offset=None, in_=xsrc_d[:],
                            in_offset=ioff)
                return at, bt

            gtiles = {0: gather(0, True), 1: gather(1, True)}
            tiles = {}

            def resolve_ab(i):
                """(a_ap, b_ap, base) for mix i; base = partition offset
                the u/w/q tmp lanes must live at (to match b for TT)."""
                op = _SCHED[i]
                l, key = op['level'], op['key']
                if l == 0:
                    at, bt = gtiles[key]
                    return at[:], bt[:], 0
                if l < 5:
                    return (tiles[(l - 1, 2 * key)][:],
                            tiles[(l - 1, 2 * key + 1)][:], 0)
                if l == 5:
                    a5, b5 = tiles['A5'], tiles['B5']
                    return a5[:], b5[:], 0
                t5 = tiles[(5, 0)]
                return t5[0:32, :], t5[32:64, :], 32

            def coefs(i, n):
                return [coef_t[0:n, 4 * i + j:4 * i + j + 1]
                        for j in range(4)]

            def w_on_sce(i):
                op = _SCHED[i]
                return (op['drain'] and op['level'] == 4) or \
                    (not op['drain'] and not op['hsplit']
                     and i % W_SCE_EVERY == W_SCE_EVERY - 1)

            def emit_u(i):
                """Pre-emit mix i's u-op (and its w-op when it rides ScalarE)
                — the lookahead keeps ScE a mix ahead of DVE."""
                op = _SCHED[i]
                n = op['lanes']
                a_ap, _, base = resolve_ab(i)
                c3, c2, c1, c0 = coefs(i, n)
                u_t = ptmp.tile([128, FRUN], f16, tag="u", name=f"u{i}")
                u_ap = u_t[base:base + n, :]
                chunks = HALVES if op['hsplit'] else FULL
                for (r0, r1) in chunks:
                    uv, av = vvr(u_ap, r0, r1), vvr(a_ap, r0, r1)
                    if op['drain'] and op['level'] >= 4:
                        nc.vector.tensor_scalar(uv, av, c3, c2,
                                                AL.mult, AL.add)
                    else:
                        nc.scalar.activation(uv, av, ACTF.Identity,
                                             bias=c2, scale=c3)
                return u_t, u_ap, None

            def can_lookahead(i):
                if i + 1 >= _NMIX:
                    return False
                nxt = _SCHED[i + 1]
                if nxt['level'] == 5:
                    return False   # A5/B5 not realigned yet
                if nxt['level'] == 0:
                    return True
                a_child = ((5, 0) if nxt['level'] == 6 else
                           (nxt['level'] - 1, 2 * nxt['key']))
                cur = _SCHED[i]
                return a_child != (cur['level'], cur['key'])

            pend_accum = []
            pending_u = None
            for i, op in enumerate(_SCHED):
                l, key, n = op['level'], op['key'], op['lanes']
                _, _, c1, c0 = coefs(i, n)

                if l == 0 and key + 2 < 16:
                    gtiles[key + 2] = gather(key + 2)
                if pend_accum:
                    rp, up = pend_accum.pop(0)
                    nc.gpsimd.dma_start(out=r3d(rp[:]), in_=r3d(up[:]),
                                        accum_op=AL.add)
                if l == 5:
                    # realign nodes {0,2} / {1,3} of T4 into A5 / B5;
                    # A5 first (gates u5), spread over both HWDGE queues
                    t4 = tiles[(4, 0)]
                    a5 = pfin.tile([64, FRUN], f16, tag="A5", name="a5")
                    b5 = pfin.tile([64, FRUN], f16, tag="B5", name="b5")
                    for (r0, r1) in HALVES:
                        sl = slice(128 * r0, 128 * r1)
                        nc.sync.dma_start(a5[0:32, sl], t4[0:32, sl])
                        nc.scalar.dma_start(a5[32:64, sl], t4[64:96, sl])
                        nc.sync.dma_start(b5[0:32, sl], t4[32:64, sl])
                        nc.scalar.dma_start(b5[32:64, sl], t4[96:128, sl])
                    tiles['A5'] = a5
                    tiles['B5'] = b5

                a_ap, b_ap, base = resolve_ab(i)
                if pending_u is not None and pending_u[0] == i:
                    u_t, u_ap, w_ahead = pending_u[1]
                else:
                    u_t, u_ap, w_ahead = emit_u(i)
                pending_u = None

                # output tile
                if l == 0:
                    r_t = pt0.tile([128, FRUN], f16, tag="T0",
                                   name=f"t0_{key}")
                elif l == 1:
                    r_t = plv1.tile([128, FRUN], f16, tag="T1",
                                    name=f"t1_{key}")
                elif l < 4:
                    r_t = plv2.tile([128, FRUN], f16, tag=f"T{l}",
                                    name=f"t{l}_{key}")
                elif l == 4:
                    r_t = pt4.tile([128, FRUN], f16, tag="T4", name="t4")
                else:
                    r_t = pfin.tile([n, FRUN], f16, tag=f"T{l}",
                                    name=f"t{l}")
                tiles[(l, key)] = r_t

                accum = (ACCUM_L0 and l == 0 and key % 2 == 0
                         and not op['hsplit'])
                if accum:
                    w_ap = r_t[:]
                elif w_ahead is not None:
                    w_ap = w_ahead
                else:
                    w_t = ptmp.tile([128, FRUN], f16, tag="w", name=f"w{i}")
                    w_ap = w_t[base:base + n, :]

                chunks = HALVES if op['hsplit'] else FULL
                for (r0, r1) in chunks:
                    av, bv = vvr(a_ap, r0, r1), vvr(b_ap, r0, r1)
                    uv, wv = vvr(u_ap, r0, r1), vvr(w_ap, r0, r1)
                    rv = vvr(r_t[:], r0, r1)
                    if w_on_sce(i):
                        nc.scalar.activation(wv, av, ACTF.Identity,
                                             bias=c0, scale=c1)
                    else:
                        nc.vector.tensor_scalar(wv, av, c1, c0,
                                                AL.mult, AL.add)
                    nc.vector.tensor_tensor(uv, uv, bv, AL.mult)
                    if (r0, r1) == chunks[-1] and can_lookahead(i):
                        pending_u = (i + 1, emit_u(i + 1))
                    if not accum:
                        nc.vector.tensor_tensor(rv, uv, wv, AL.add)
                        if l == 6:
                            sl = slice(128 * r0, 128 * r1)
                            nc.sync.dma_start(out_d[:, sl], r_t[:, sl])
                if accum:
                    pend_accum.append((r_t, u_t))
    nc.compile()
    _BASS_CACHE['nc'] = nc
    return nc


def _prep_inputs(x, idx_a, idx_b, ws):
    x = np.ascontiguousarray(x, dtype=np.float32)
    in_maps = []
    for core in range(NCORES):
        g, h = core % 4, core // 4
        coef = _coef_tables(ws, h)
        offs = _offset_tables(idx_a, idx_b, h)
        # b4-interleaved slice: [C,H,W,B4] fp16
        xs = x[B4 * g:B4 * g + B4].transpose(1, 2, 3, 0)
        xp = np.zeros((XPAD,), dtype=np.float16)
        xp[:B4 * C * H * W] = xs.reshape(-1).astype(np.float16)
        in_maps.append({"xsrc": xp.reshape(XPAD, 1), "offs": offs,
                        "coef": coef})
    return in_maps


def _assemble(core_outs):
    """core_outs: list of [32, FRUN] f16 -> [16,64,900,1] f32."""
    full = np.empty((B, K, P, 1), dtype=np.float32)
    for core, o in enumerate(core_outs):
        g, h = core % 4, core // 4
        v = np.asarray(o).reshape(KH, PW, 32, B4)[:, :, 0:PW, :]  # k,hh,ww,b
        v = v.astype(np.float32).transpose(3, 0, 1, 2)            # b,k,hh,ww
        full[B4 * g:B4 * g + B4, KH * h:KH * h + KH] = \
            v.reshape(B4, KH, P, 1)
    return np.ascontiguousarray(full)


def kernel(x, idx_a, idx_b, w0, w1, w2, w3, w4, w5, w6):
    ws = [np.asarray(w, dtype=np.float32) for w in
          (w0, w1, w2, w3, w4, w5, w6)]
    x = np.asarray(x, dtype=np.float32)
    idx_a = np.asarray(idx_a, dtype=np.int32)
    idx_b = np.asarray(idx_b, dtype=np.int32)
    in_maps = _prep_inputs(x, idx_a, idx_b, ws)
    nc = _build_bass()
    from concourse.bass_utils import run_bass_kernel_spmd
    res = run_bass_kernel_spmd(nc, in_maps, core_ids=list(range(NCORES)))
    return _assemble([r["out"] for r in res.results])


def kernel_emulate(x, idx_a, idx_b, w0, w1, w2, w3, w4, w5, w6):
    """Pure-numpy emulation of the exact device schedule (debug aid)."""
    ws = [np.asarray(w, dtype=np.float32) for w in
          (w0, w1, w2, w3, w4, w5, w6)]
    in_maps = _prep_inputs(np.asarray(x, np.float32),
                           np.asarray(idx_a, np.int32),
                           np.asarray(idx_b, np.int32), ws)
    outs = [_emulate_core(m["xsrc"].reshape(-1), m["offs"], m["coef"])
            for m in in_maps]
    return _assemble(outs)


# revision 44
# speedup vs baseline: 1.0709x; 1.0056x over previous
"""Trainium2 Bass kernel for nn_LogicConv3d (differentiable-logic conv tree).

Problem (hardcoded): x [16,64,32,32] f32; idx_a/idx_b [64,900,64,3] i32;
w0..w6 [s,64,16] f32 (s = 64,32,16,8,4,2,1). Output [16,64,900,1] f32.

Math: per (kernel k, window p): gather 64 (a,b) leaf pairs from x, blend each
pair with soft-gate coefficients (softmax(w) @ GATE_M), then 6 more pairwise
tree levels.  mix(a,b) = c0 + c1*a + c2*b + c3*a*b.

Mapping (fp16, ~195us vs 293us f32 baseline):
 - Sharding: 8 cores = 4 batch-groups x 2 kernel-halves.  Core c handles
   batches [4g,4g+4) (g=c%4) and kernels [32h,32h+32) (h=c//4).  The device
   program is identical across cores; tables and the x-slice differ.
 - x-slice is stored fp16, b4-interleaved ([C,H,W,4] flat).  Each leaf
   (node,kern) lane gathers ONE contiguous 3840-elem run (30 rows x 32 cols
   x 4 batches) via indirect DMA -> [128, 3840] tile; cols w=30,31 are junk
   (row wrap) and simply never touched by the valid views.
 - All tree tensors fp16.  Mix is computed as
       u = c3*a + c2   (ScalarE ACT, per-lane scale/bias; emitted one mix
                        AHEAD so ScE never gates the DVE q-op)
       w = c1*a + c0   (DVE tensor_scalar 4x fp16, every 3rd on ScalarE)
       q = u * b       (DVE tensor_tensor, 2x fp16, in-place over u)
       r = q + w       (DVE tensor_tensor, 2x fp16)
   which is exact (= c0 + c1*a + c2*b + c3*ab) with NO constant folding and
   avoids scalar_tensor_tensor entirely (stt has no 2x uop -> 1 elem/cycle).
   The kernel is DVE-throughput-bound (~160us busy at ~92% occupancy).
 - Partition layout: lane = 32*u + kern_local, node = key + (16>>level)*u.
   Merges are delayed one leaf-pair behind their producers (software
   pipelining).  L5 needs a partition realignment of T4 (TT requires equal
   input base partitions): 4 SBUF->SBUF DMAs split over both HWDGE queues.
   L6 avoids realignment by writing its u/w at partition base 32 (single-
   input ops may shift partition base; TT inputs must only match each other).
 - Rejected via measurement: DMA CCE-accum r-offload (3x port traffic makes
   the leaf phase DMA-bound), CCE mult (verifier rejects), GPSIMD elementwise
   (shares the 2nd DVE SBUF port), W_SCE_EVERY=2 (overloads ScalarE),
   quarter-granularity chunking (per-op overhead exceeds pipelining gain).
"""
import numpy as np

B, C, H, W = 16, 64, 32, 32
K = 64
RF = 3
DEPTH = 6
S = 64
PW = 30            # windows per axis
P = PW * PW        # 900
NCORES = 8
B4 = 4             # batches per core
KH = 32            # kernels per core
FRUN = 30 * 32 * B4          # 3840: gather run / tile free size
XPAD = C * H * W * B4 + 4096  # fp16 elements in the padded x slice

GATE_M = np.array([
    [0, 0, 0, 0], [0, 0, 0, 1], [0, 1, 0, -1], [0, 1, 0, 0],
    [0, 0, 1, -1], [0, 0, 1, 0], [0, 1, 1, -2], [0, 1, 1, -1],
    [1, -1, -1, 1], [1, -1, -1, 2], [1, 0, -1, 0], [1, 0, -1, 1],
    [1, -1, 0, 0], [1, -1, 0, 1], [1, 0, 0, -1], [1, 0, 0, 0],
], dtype=np.float32)  # [16 gates, 4] -> c0,c1,c2,c3 = GATE_M.T @ softmax(w)

W_SCE_EVERY = 3    # every 3rd mix puts its w-op on ScalarE (engine balance)
ACCUM_L0 = False   # CCE-accum r-offload: measured net-negative (the 3x
                   # SBUF-port traffic of read-modify-write DMAs makes the
                   # leaf phase DMA-bound; DVE savings don't pay for it)


# ---------------------------------------------------------------------------
# static schedule
# ---------------------------------------------------------------------------
def _build_schedule():
    """Software-pipelined order over the merge tree.  Each entry:
    dict(level, key, lanes, node[lanes], hsplit) with lane = 32*u + kern,
    node = key + (16>>l)*u for l<=4; L5: two 32-lane ops (node=key);
    L6: 32 lanes node=0.  Merges are delayed by one leaf-pair so that
    producers finish well before consumers (hides DMA-accum latency).
    Ops after the last leaf (the drain chain) are marked hsplit for
    row-chunked tail pipelining."""
    from collections import deque

    def mk(l, key):
        if l <= 4:
            lanes = np.arange(128)
            node = key + (16 >> l) * (lanes >> 5)
        elif l == 5:
            node = np.arange(64) >> 5
        else:
            node = np.zeros(32, np.int64)
        return dict(level=l, key=key, lanes=len(node), node=node,
                    hsplit=False)

    children = {}
    for l in range(1, 5):
        for k in range(16 >> l):
            children[(l, k)] = [(l - 1, 2 * k), (l - 1, 2 * k + 1)]
    children[(5, 0)] = [(4, 0)]
    children[(6, 0)] = [(5, 0)]
    parents = {}
    for p, cs in children.items():
        for c in cs:
            parents.setdefault(c, []).append(p)

    order = []
    emitted = set()
    ready = deque()

    def emit(node):
        order.append(node)
        emitted.add(node)
        for p in parents.get(node, []):
            if p not in emitted and p not in ready and \
                    all(c in emitted for c in children[p]):
                ready.append(p)

    for pair in range(8):
        snap = len(ready)
        emit((0, 2 * pair))
        emit((0, 2 * pair + 1))
        for _ in range(snap):
            emit(ready.popleft())
    n_before_drain = len(order)
    while ready:
        emit(ready.popleft())

    ops = [mk(l, k) for (l, k) in order]
    for i in range(n_before_drain, len(ops)):
        ops[i]['drain'] = True
    for op in ops:
        op.setdefault('drain', False)
    ops[0]['hsplit'] = True   # fast head: overlap first gathers w/ compute
    ops[1]['hsplit'] = True
    return ops


_SCHED = _build_schedule()
_NMIX = len(_SCHED)          # 33
_NCOLS = 4 * _NMIX


def _softmax_f32(w):
    w = w.astype(np.float64)
    m = w.max(-1, keepdims=True)
    e = np.exp(w - m)
    return e / e.sum(-1, keepdims=True)


def _coef_tables(ws, khalf):
    """ws = [w0..w6]; khalf = kernel half (0/1).  Returns [128, _NCOLS] f32:
    cols 4i..4i+3 = (c3, c2, c1, c0) of mix i for this lane's (node, kern)."""
    cs = []
    for wl in ws:
        p = _softmax_f32(wl)                      # [s, K, 16] f64
        cs.append(np.einsum('skg,gj->skj', p, GATE_M.astype(np.float64)))
    coef = np.zeros((128, _NCOLS), dtype=np.float64)
    for i, op in enumerate(_SCHED):
        l, node, n = op['level'], op['node'], op['lanes']
        kern = KH * khalf + (np.arange(n) & 31)
        c = cs[l][node, kern]                     # [n, 4] = (c0,c1,c2,c3)
        coef[np.arange(n), 4 * i + 0] = c[:, 3]
        coef[np.arange(n), 4 * i + 1] = c[:, 2]
        coef[np.arange(n), 4 * i + 2] = c[:, 1]
        coef[np.arange(n), 4 * i + 3] = c[:, 0]
    return coef.astype(np.float32)


def _offset_tables(idx_a, idx_b, khalf):
    """Gather index tables [128, 32] i32: col = 2*t + side.
    Element offsets into the b4-interleaved fp16 x-slice."""
    offs = np.zeros((128, 32), dtype=np.int64)
    for op in _SCHED:
        if op['level'] != 0:
            continue
        t = op['key']
        kern = KH * khalf + (np.arange(128) & 31)
        for side, idx in ((0, idx_a), (1, idx_b)):
            ha = idx[kern, 0, op['node'], 0].astype(np.int64)
            wa = idx[kern, 0, op['node'], 1].astype(np.int64)
            ca = idx[kern, 0, op['node'], 2].astype(np.int64)
            offs[:, 2 * t + side] = (ca * (H * W) + ha * W + wa) * B4
    return offs.astype(np.int32)


# ---------------------------------------------------------------------------
# numpy emulator (mirrors the device schedule exactly; for validation)
# ---------------------------------------------------------------------------
def _emulate_core(xp, offs, coef):
    """xp: [XPAD] f16 slice. Returns [32, FRUN] f16 (junk cols included)."""
    f16, f32 = np.float16, np.float32
    tiles = {}

    def mix(i, a, b, n):
        c3 = coef[np.arange(n), 4 * i + 0][:, None].astype(f32)
        c2 = coef[np.arange(n), 4 * i + 1][:, None].astype(f32)
        c1 = coef[np.arange(n), 4 * i + 2][:, None].astype(f32)
        c0 = coef[np.arange(n), 4 * i + 3][:, None].astype(f32)
        u = (a.astype(f32) * c3 + c2).astype(f16)
        w = (a.astype(f32) * c1 + c0).astype(f16)
        q = (u.astype(f32) * b.astype(f32)).astype(f16)
        return (q.astype(f32) + w.astype(f32)).astype(f16)

    for i, op in enumerate(_SCHED):
        l, key, n = op['level'], op['key'], op['lanes']
        if l == 0:
            o = offs[:, 2 * key + 0]
            a = xp[o[:, None] + np.arange(FRUN)[None, :]]
            o = offs[:, 2 * key + 1]
            b = xp[o[:, None] + np.arange(FRUN)[None, :]]
        elif l < 5:
            a = tiles[(l - 1, 2 * key)]
            b = tiles[(l - 1, 2 * key + 1)]
        elif l == 5:
            t4 = tiles[(4, 0)]
            a = np.concatenate([t4[0:32], t4[64:96]])     # nodes 0,2
            b = np.concatenate([t4[32:64], t4[96:128]])   # nodes 1,3
        else:
            t5 = tiles[(5, 0)]
            a, b = t5[0:32], t5[32:64]
        tiles[(l, key)] = mix(i, a, b, n)
    return tiles[(6, 0)]


# ---------------------------------------------------------------------------
# Bass program (built once, cached)
# ---------------------------------------------------------------------------
_BASS_CACHE = {}


def _build_bass():
    if 'nc' in _BASS_CACHE:
        return _BASS_CACHE['nc']
    import concourse.bass as bass
    import concourse.mybir as mybir
    import concourse.tile as tile
    import concourse.bacc as bacc

    f16 = mybir.dt.float16
    f32 = mybir.dt.float32
    nc = bacc.Bacc("TRN2", target_bir_lowering=False, debug=False,
                   num_devices=NCORES)
    xsrc_d = nc.dram_tensor("xsrc", [XPAD, 1], f16, kind="ExternalInput").ap()
    offs_d = nc.dram_tensor("offs", [128, 32], mybir.dt.int32,
                            kind="ExternalInput").ap()
    coef_d = nc.dram_tensor("coef", [128, _NCOLS], f32,
                            kind="ExternalInput").ap()
    out_d = nc.dram_tensor("out", [32, FRUN], f16, kind="ExternalOutput").ap()

    AL = mybir.AluOpType
    ACTF = mybir.ActivationFunctionType

    HF = FRUN // 2     # 1920: h-half size

    def vvr(ap, r0, r1):
        # valid view of rows [r0,r1) of a [n, FRUN] AP (skip w=30,31 junk)
        return ap.rearrange("p (h wb) -> p h wb", h=30, wb=128)[
            :, r0:r1, 0:120]

    def r3d(ap):
        # [n, FRUN] -> [n, 2, 1920] (CCE descriptor length <= 2048 elems)
        return ap.rearrange("p (a b) -> p a b", a=2, b=HF)

    HALVES = [(0, 15), (15, 30)]
    QUARTERS = [(0, 8), (8, 15), (15, 23), (23, 30)]
    FULL = [(0, 30)]

    with tile.TileContext(nc) as tc:
        with (
            tc.tile_pool(name="const", bufs=1) as pc,
            tc.tile_pool(name="ab", bufs=3) as pab,
            tc.tile_pool(name="t0p", bufs=4) as pt0,
            tc.tile_pool(name="lv1", bufs=3) as plv1,
            tc.tile_pool(name="lv2", bufs=2) as plv2,
            tc.tile_pool(name="t4p", bufs=1) as pt4,
            tc.tile_pool(name="tmp", bufs=2) as ptmp,
            tc.tile_pool(name="fin", bufs=1) as pfin,
        ):
            offs_t = pc.tile([128, 32], mybir.dt.int32, tag="offs",
                             name="offs_t")
            nc.gpsimd.dma_start(offs_t[:], offs_d[:])
            coef_t = pc.tile([128, _NCOLS], f32, tag="coef", name="coef_t")
            nc.sync.dma_start(coef_t[:], coef_d[:])
            warm_t = pc.tile([1, 8], f32, tag="warm", name="warm_t")
            nc.scalar.activation(warm_t[:], coef_t[0:1, 0:8],
                                 ACTF.Identity, bias=0.0, scale=1.0)

            def gather(t, split=False):
                at = pab.tile([128, FRUN], f16, tag="A", name=f"a{t}")
                bt = pab.tile([128, FRUN], f16, tag="B", name=f"b{t}")
                ioffs = [bass.IndirectOffsetOnAxis(
                    ap=offs_t[:, 2 * t + s:2 * t + s + 1], axis=0)
                    for s in (0, 1)]
                if split:
                    # interleave A/B halves so the first q (needs A-h0 AND
                    # B-h0) unblocks one dispatch earlier
                    for (r0, r1) in HALVES:
                        for dst, ioff in ((at, ioffs[0]), (bt, ioffs[1])):
                            nc.gpsimd.indirect_dma_start(
                                out=dst[:, 128 * r0:128 * r1],
                                out_offset=None, in_=xsrc_d[:],
                                in_offset=ioff, element_offset=128 * r0)
                else:
                    for dst, ioff in ((at, ioffs[0]), (bt, ioffs[1])):
                        nc.gpsimd.indirect_dma_start(
                            out=dst[:], out_offset=None, in_=xsrc_d[:],
                            in_offset=ioff)
                return at, bt

            gtiles = {0: gather(0, True), 1: gather(1, True)}
            tiles = {}

            def resolve_ab(i):
                """(a_ap, b_ap, base) for mix i; base = partition offset
                the u/w/q tmp lanes must live at (to match b for TT)."""
                op = _SCHED[i]
                l, key = op['level'], op['key']
                if l == 0:
                    at, bt = gtiles[key]
                    return at[:], bt[:], 0
                if l < 5:
                    return (tiles[(l - 1, 2 * key)][:],
                            tiles[(l - 1, 2 * key + 1)][:], 0)
                if l == 5:
                    a5, b5 = tiles['A5'], tiles['B5']
                    return a5[:], b5[:], 0
                t5 = tiles[(5, 0)]
                return t5[0:32, :], t5[32:64, :], 32

            def coefs(i, n):
                return [coef_t[0:n, 4 * i + j:4 * i + j + 1]
                        for j in range(4)]

            def w_on_sce(i):
                op = _SCHED[i]
                return (op['drain'] and op['level'] == 4) or \
                    (not op['drain'] and not op['hsplit']
                     and i % W_SCE_EVERY == W_SCE_EVERY - 1)

            def emit_u(i):
                """Pre-emit mix i's u-op (and its w-op when it rides ScalarE)
                — the lookahead keeps ScE a mix ahead of DVE."""
                op = _SCHED[i]
                n = op['lanes']
                a_ap, _, base = resolve_ab(i)
                c3, c2, c1, c0 = coefs(i, n)
                u_t = ptmp.tile([128, FRUN], f16, tag="u", name=f"u{i}")
                u_ap = u_t[base:base + n, :]
                chunks = HALVES if op['hsplit'] else FULL
                for (r0, r1) in chunks:
                    uv, av = vvr(u_ap, r0, r1), vvr(a_ap, r0, r1)
                    if (op['drain'] and op['level'] >= 4) or op['hsplit']:
                        # drain tail and head mixes: keep the u->q chain
                        # on DVE (no cross-engine semaphore hop)
                        nc.vector.tensor_scalar(uv, av, c3, c2,
                                                AL.mult, AL.add)
                    else:
                        nc.scalar.activation(uv, av, ACTF.Identity,
                                             bias=c2, scale=c3)
                return u_t, u_ap, None

            def can_lookahead(i):
                if i + 1 >= _NMIX:
                    return False
                nxt = _SCHED[i + 1]
                if nxt['level'] == 5:
                    return False   # A5/B5 not realigned yet
                if nxt['level'] == 0:
                    return True
                a_child = ((5, 0) if nxt['level'] == 6 else
                           (nxt['level'] - 1, 2 * nxt['key']))
                cur = _SCHED[i]
                return a_child != (cur['level'], cur['key'])

            pend_accum = []
            pending_u = None
            for i, op in enumerate(_SCHED):
                l, key, n = op['level'], op['key'], op['lanes']
                _, _, c1, c0 = coefs(i, n)

                if l == 0 and key + 2 < 16:
                    gtiles[key + 2] = gather(key + 2)
                if pend_accum:
                    rp, up = pend_accum.pop(0)
                    nc.gpsimd.dma_start(out=r3d(rp[:]), in_=r3d(up[:]),
                                        accum_op=AL.add)
                if l == 5:
                    # realign nodes {0,2} / {1,3} of T4 into A5 / B5;
                    # A5 first (gates u5), spread over both HWDGE queues
                    t4 = tiles[(4, 0)]
                    a5 = pfin.tile([64, FRUN], f16, tag="A5", name="a5")
                    b5 = pfin.tile([64, FRUN], f16, tag="B5", name="b5")
                    nc.sync.dma_start(a5[0:32, :], t4[0:32, :])
                    nc.scalar.dma_start(a5[32:64, :], t4[64:96, :])
                    nc.sync.dma_start(b5[0:32, :], t4[32:64, :])
                    nc.scalar.dma_start(b5[32:64, :], t4[96:128, :])
                    tiles['A5'] = a5
                    tiles['B5'] = b5

                a_ap, b_ap, base = resolve_ab(i)
                if pending_u is not None and pending_u[0] == i:
                    u_t, u_ap, w_ahead = pending_u[1]
                else:
                    u_t, u_ap, w_ahead = emit_u(i)
                pending_u = None

                # output tile
                if l == 0:
                    r_t = pt0.tile([128, FRUN], f16, tag="T0",
                                   name=f"t0_{key}")
                elif l == 1:
                    r_t = plv1.tile([128, FRUN], f16, tag="T1",
                                    name=f"t1_{key}")
                elif l < 4:
                    r_t = plv2.tile([128, FRUN], f16, tag=f"T{l}",
                                    name=f"t{l}_{key}")
                elif l == 4:
                    r_t = pt4.tile([128, FRUN], f16, tag="T4", name="t4")
                else:
                    r_t = pfin.tile([n, FRUN], f16, tag=f"T{l}",
                                    name=f"t{l}")
                tiles[(l, key)] = r_t

                accum = (ACCUM_L0 and l == 0 and key % 2 == 0
                         and not op['hsplit'])
                if accum:
                    w_ap = r_t[:]
                elif w_ahead is not None:
                    w_ap = w_ahead
                else:
                    w_t = ptmp.tile([128, FRUN], f16, tag="w", name=f"w{i}")
                    w_ap = w_t[base:base + n, :]

                chunks = HALVES if op['hsplit'] else FULL
                for (r0, r1) in chunks:
                    av, bv = vvr(a_ap, r0, r1), vvr(b_ap, r0, r1)
                    uv, wv = vvr(u_ap, r0, r1), vvr(w_ap, r0, r1)
                    rv = vvr(r_t[:], r0, r1)
                    if w_on_sce(i):
                        nc.scalar.activation(wv, av, ACTF.Identity,
                                             bias=c0, scale=c1)
                    else:
                        nc.vector.tensor_scalar(wv, av, c1, c0,
                                                AL.mult, AL.add)
                    nc.vector.tensor_tensor(uv, uv, bv, AL.mult)
                    if (r0, r1) == chunks[-1] and can_lookahead(i):
                        pending_u = (i + 1, emit_u(i + 1))
                    if not accum:
                        nc.vector.tensor_tensor(rv, uv, wv, AL.add)
                        if l == 6:
                            sl = slice(128 * r0, 128 * r1)
                            nc.sync.dma_start(out_d[:, sl], r_t[:, sl])
                if accum:
                    pend_accum.append((r_t, u_t))
    nc.compile()
    _BASS_CACHE['nc'] = nc
    return nc


def _prep_inputs(x, idx_a, idx_b, ws):
    x = np.ascontiguousarray(x, dtype=np.float32)
    in_maps = []
    for core in range(NCORES):
        g, h = core % 4, core // 4
        coef = _coef_tables(ws, h)
        offs = _offset_tables(idx_a, idx_b, h)
        # b4-interleaved slice: [C,H,W,B4] fp16
        xs = x[B4 * g:B4 * g + B4].transpose(1, 2, 3, 0)
        xp = np.zeros((XPAD,), dtype=np.float16)
        xp[:B4 * C * H * W] = xs.reshape(-1).astype(np.float16)
        in_maps.append({"xsrc": xp.reshape(XPAD, 1), "offs": offs,
                        "coef": coef})
    return in_maps


def _assemble(core_outs):
    """core_outs: list of [32, FRUN] f16 -> [16,64,900,1] f32."""
    full = np.empty((B, K, P, 1), dtype=np.float32)
    for core, o in enumerate(core_outs):
        g, h = core % 4, core // 4
        v = np.asarray(o).reshape(KH, PW, 32, B4)[:, :, 0:PW, :]  # k,hh,ww,b
        v = v.astype(np.float32).transpose(3, 0, 1, 2)            # b,k,hh,ww
        full[B4 * g:B4 * g + B4, KH * h:KH * h + KH] = \
            v.reshape(B4, KH, P, 1)
    return np.ascontiguousarray(full)


def kernel(x, idx_a, idx_b, w0, w1, w2, w3, w4, w5, w6):
    ws = [np.asarray(w, dtype=np.float32) for w in
          (w0, w1, w2, w3, w4, w5, w6)]
    x = np.asarray(x, dtype=np.float32)
    idx_a = np.asarray(idx_a, dtype=np.int32)
    idx_b = np.asarray(idx_b, dtype=np.int32)
    in_maps = _prep_inputs(x, idx_a, idx_b, ws)
    nc = _build_bass()
    from concourse.bass_utils import run_bass_kernel_spmd
    res = run_bass_kernel_spmd(nc, in_maps, core_ids=list(range(NCORES)))
    return _assemble([r["out"] for r in res.results])


def kernel_emulate(x, idx_a, idx_b, w0, w1, w2, w3, w4, w5, w6):
    """Pure-numpy emulation of the exact device schedule (debug aid)."""
    ws = [np.asarray(w, dtype=np.float32) for w in
          (w0, w1, w2, w3, w4, w5, w6)]
    in_maps = _prep_inputs(np.asarray(x, np.float32),
                           np.asarray(idx_a, np.int32),
                           np.asarray(idx_b, np.int32), ws)
    outs = [_emulate_core(m["xsrc"].reshape(-1), m["offs"], m["coef"])
            for m in in_maps]
    return _assemble(outs)


# revision 53
# speedup vs baseline: 1.0781x; 1.0067x over previous
"""Trainium2 Bass kernel for nn_LogicConv3d (differentiable-logic conv tree).

Problem (hardcoded): x [16,64,32,32] f32; idx_a/idx_b [64,900,64,3] i32;
w0..w6 [s,64,16] f32 (s = 64,32,16,8,4,2,1). Output [16,64,900,1] f32.

Math: per (kernel k, window p): gather 64 (a,b) leaf pairs from x, blend each
pair with soft-gate coefficients (softmax(w) @ GATE_M), then 6 more pairwise
tree levels.  mix(a,b) = c0 + c1*a + c2*b + c3*a*b.

Mapping (fp16, ~195us vs 293us f32 baseline):
 - Sharding: 8 cores = 4 batch-groups x 2 kernel-halves.  Core c handles
   batches [4g,4g+4) (g=c%4) and kernels [32h,32h+32) (h=c//4).  The device
   program is identical across cores; tables and the x-slice differ.
 - x-slice is stored fp16, b4-interleaved ([C,H,W,4] flat).  Each leaf
   (node,kern) lane gathers ONE contiguous 3840-elem run (30 rows x 32 cols
   x 4 batches) via indirect DMA -> [128, 3840] tile; cols w=30,31 are junk
   (row wrap) and simply never touched by the valid views.
 - All tree tensors fp16.  Mix is computed as
       u = c3*a + c2   (ScalarE ACT, per-lane scale/bias; emitted one mix
                        AHEAD so ScE never gates the DVE q-op)
       w = c1*a + c0   (DVE tensor_scalar 4x fp16, every 3rd on ScalarE)
       q = u * b       (DVE tensor_tensor, 2x fp16, in-place over u)
       r = q + w       (DVE tensor_tensor, 2x fp16)
   which is exact (= c0 + c1*a + c2*b + c3*ab) with NO constant folding and
   avoids scalar_tensor_tensor entirely (stt has no 2x uop -> 1 elem/cycle).
   The kernel is DVE-throughput-bound (~160us busy at ~92% occupancy).
 - Partition layout: lane = 32*u + kern_local, node = key + (16>>level)*u.
   Merges are delayed one leaf-pair behind their producers (software
   pipelining).  L5 needs a partition realignment of T4 (TT requires equal
   input base partitions): 4 SBUF->SBUF DMAs split over both HWDGE queues.
   L6 avoids realignment by writing its u/w at partition base 32 (single-
   input ops may shift partition base; TT inputs must only match each other).
 - Rejected via measurement: DMA CCE-accum r-offload (3x port traffic makes
   the leaf phase DMA-bound), CCE mult (verifier rejects), GPSIMD elementwise
   (shares the 2nd DVE SBUF port), W_SCE_EVERY=2 (overloads ScalarE),
   quarter-granularity chunking (per-op overhead exceeds pipelining gain).
"""
import numpy as np

B, C, H, W = 16, 64, 32, 32
K = 64
RF = 3
DEPTH = 6
S = 64
PW = 30            # windows per axis
P = PW * PW        # 900
NCORES = 8
B4 = 4             # batches per core
KH = 32            # kernels per core
FRUN = 30 * 32 * B4          # 3840: gather run / tile free size
XPAD = C * H * W * B4 + 4096  # fp16 elements in the padded x slice

GATE_M = np.array([
    [0, 0, 0, 0], [0, 0, 0, 1], [0, 1, 0, -1], [0, 1, 0, 0],
    [0, 0, 1, -1], [0, 0, 1, 0], [0, 1, 1, -2], [0, 1, 1, -1],
    [1, -1, -1, 1], [1, -1, -1, 2], [1, 0, -1, 0], [1, 0, -1, 1],
    [1, -1, 0, 0], [1, -1, 0, 1], [1, 0, 0, -1], [1, 0, 0, 0],
], dtype=np.float32)  # [16 gates, 4] -> c0,c1,c2,c3 = GATE_M.T @ softmax(w)

W_SCE_EVERY = 3    # every 3rd mix puts its w-op on ScalarE (engine balance)
ACCUM_L0 = False   # CCE-accum r-offload: measured net-negative (the 3x
                   # SBUF-port traffic of read-modify-write DMAs makes the
                   # leaf phase DMA-bound; DVE savings don't pay for it)


# ---------------------------------------------------------------------------
# static schedule
# ---------------------------------------------------------------------------
def _build_schedule():
    """Software-pipelined order over the merge tree.  Each entry:
    dict(level, key, lanes, node[lanes], hsplit) with lane = 32*u + kern,
    node = key + (16>>l)*u for l<=4; L5: two 32-lane ops (node=key);
    L6: 32 lanes node=0.  Merges are delayed by one leaf-pair so that
    producers finish well before consumers (hides DMA-accum latency).
    Ops after the last leaf (the drain chain) are marked hsplit for
    row-chunked tail pipelining."""
    from collections import deque

    def mk(l, key):
        if l <= 4:
            lanes = np.arange(128)
            node = key + (16 >> l) * (lanes >> 5)
        elif l == 5:
            node = np.full(32, key, np.int64)
        else:
            node = np.zeros(32, np.int64)
        return dict(level=l, key=key, lanes=len(node), node=node,
                    hsplit=False)

    children = {}
    for l in range(1, 5):
        for k in range(16 >> l):
            children[(l, k)] = [(l - 1, 2 * k), (l - 1, 2 * k + 1)]
    children[(5, 0)] = [(4, 0)]
    children[(5, 1)] = [(4, 0)]
    children[(6, 0)] = [(5, 0), (5, 1)]
    parents = {}
    for p, cs in children.items():
        for c in cs:
            parents.setdefault(c, []).append(p)

    order = []
    emitted = set()
    ready = deque()

    def emit(node):
        order.append(node)
        emitted.add(node)
        for p in parents.get(node, []):
            if p not in emitted and p not in ready and \
                    all(c in emitted for c in children[p]):
                ready.append(p)

    for pair in range(8):
        snap = len(ready)
        emit((0, 2 * pair))
        emit((0, 2 * pair + 1))
        for _ in range(snap):
            emit(ready.popleft())
    n_before_drain = len(order)
    while ready:
        emit(ready.popleft())

    ops = [mk(l, k) for (l, k) in order]
    for i in range(n_before_drain, len(ops)):
        ops[i]['drain'] = True
    for op in ops:
        op.setdefault('drain', False)
    ops[0]['hsplit'] = True   # fast head: overlap first gathers w/ compute
    ops[1]['hsplit'] = True
    return ops


_SCHED = _build_schedule()
_NMIX = len(_SCHED)          # 33
_NCOLS = 4 * _NMIX


def _softmax_f32(w):
    w = w.astype(np.float64)
    m = w.max(-1, keepdims=True)
    e = np.exp(w - m)
    return e / e.sum(-1, keepdims=True)


def _coef_tables(ws, khalf):
    """ws = [w0..w6]; khalf = kernel half (0/1).  Returns [128, _NCOLS] f32:
    cols 4i..4i+3 = (c3, c2, c1, c0) of mix i for this lane's (node, kern)."""
    cs = []
    for wl in ws:
        p = _softmax_f32(wl)                      # [s, K, 16] f64
        cs.append(np.einsum('skg,gj->skj', p, GATE_M.astype(np.float64)))
    coef = np.zeros((128, _NCOLS), dtype=np.float64)
    for i, op in enumerate(_SCHED):
        l, node, n = op['level'], op['node'], op['lanes']
        kern = KH * khalf + (np.arange(n) & 31)
        c = cs[l][node, kern]                     # [n, 4] = (c0,c1,c2,c3)
        coef[np.arange(n), 4 * i + 0] = c[:, 3]
        coef[np.arange(n), 4 * i + 1] = c[:, 2]
        coef[np.arange(n), 4 * i + 2] = c[:, 1]
        coef[np.arange(n), 4 * i + 3] = c[:, 0]
    return coef.astype(np.float32)


def _offset_tables(idx_a, idx_b, khalf):
    """Gather index tables [128, 32] i32: col = 2*t + side.
    Element offsets into the b4-interleaved fp16 x-slice."""
    offs = np.zeros((128, 32), dtype=np.int64)
    for op in _SCHED:
        if op['level'] != 0:
            continue
        t = op['key']
        kern = KH * khalf + (np.arange(128) & 31)
        for side, idx in ((0, idx_a), (1, idx_b)):
            ha = idx[kern, 0, op['node'], 0].astype(np.int64)
            wa = idx[kern, 0, op['node'], 1].astype(np.int64)
            ca = idx[kern, 0, op['node'], 2].astype(np.int64)
            offs[:, 2 * t + side] = (ca * (H * W) + ha * W + wa) * B4
    return offs.astype(np.int32)


# ---------------------------------------------------------------------------
# numpy emulator (mirrors the device schedule exactly; for validation)
# ---------------------------------------------------------------------------
def _emulate_core(xp, offs, coef):
    """xp: [XPAD] f16 slice. Returns [32, FRUN] f16 (junk cols included)."""
    f16, f32 = np.float16, np.float32
    tiles = {}

    def mix(i, a, b, n):
        c3 = coef[np.arange(n), 4 * i + 0][:, None].astype(f32)
        c2 = coef[np.arange(n), 4 * i + 1][:, None].astype(f32)
        c1 = coef[np.arange(n), 4 * i + 2][:, None].astype(f32)
        c0 = coef[np.arange(n), 4 * i + 3][:, None].astype(f32)
        u = (a.astype(f32) * c3 + c2).astype(f16)
        w = (a.astype(f32) * c1 + c0).astype(f16)
        q = (u.astype(f32) * b.astype(f32)).astype(f16)
        return (q.astype(f32) + w.astype(f32)).astype(f16)

    for i, op in enumerate(_SCHED):
        l, key, n = op['level'], op['key'], op['lanes']
        if l == 0:
            o = offs[:, 2 * key + 0]
            a = xp[o[:, None] + np.arange(FRUN)[None, :]]
            o = offs[:, 2 * key + 1]
            b = xp[o[:, None] + np.arange(FRUN)[None, :]]
        elif l < 5:
            a = tiles[(l - 1, 2 * key)]
            b = tiles[(l - 1, 2 * key + 1)]
        elif l == 5:
            t4 = tiles[(4, 0)]
            a = t4[64 * key:64 * key + 32]
            b = t4[64 * key + 32:64 * key + 64]
        else:
            a, b = tiles[(5, 0)], tiles[(5, 1)]
        tiles[(l, key)] = mix(i, a, b, n)
    return tiles[(6, 0)]


# ---------------------------------------------------------------------------
# Bass program (built once, cached)
# ---------------------------------------------------------------------------
_BASS_CACHE = {}


def _build_bass():
    if 'nc' in _BASS_CACHE:
        return _BASS_CACHE['nc']
    import concourse.bass as bass
    import concourse.mybir as mybir
    import concourse.tile as tile
    import concourse.bacc as bacc

    f16 = mybir.dt.float16
    f32 = mybir.dt.float32
    nc = bacc.Bacc("TRN2", target_bir_lowering=False, debug=False,
                   num_devices=NCORES)
    xsrc_d = nc.dram_tensor("xsrc", [XPAD, 1], f16, kind="ExternalInput").ap()
    offs_d = nc.dram_tensor("offs", [128, 32], mybir.dt.int32,
                            kind="ExternalInput").ap()
    coef_d = nc.dram_tensor("coef", [128, _NCOLS], f32,
                            kind="ExternalInput").ap()
    out_d = nc.dram_tensor("out", [32, FRUN], f16, kind="ExternalOutput").ap()

    AL = mybir.AluOpType
    ACTF = mybir.ActivationFunctionType

    HF = FRUN // 2     # 1920: h-half size

    def vvr(ap, r0, r1):
        # valid view of rows [r0,r1) of a [n, FRUN] AP (skip w=30,31 junk)
        return ap.rearrange("p (h wb) -> p h wb", h=30, wb=128)[
            :, r0:r1, 0:120]

    def r3d(ap):
        # [n, FRUN] -> [n, 2, 1920] (CCE descriptor length <= 2048 elems)
        return ap.rearrange("p (a b) -> p a b", a=2, b=HF)

    HALVES = [(0, 15), (15, 30)]
    QUARTERS = [(0, 8), (8, 15), (15, 23), (23, 30)]
    FULL = [(0, 30)]

    with tile.TileContext(nc) as tc:
        with (
            tc.tile_pool(name="const", bufs=1) as pc,
            tc.tile_pool(name="ab", bufs=4) as pab,
            tc.tile_pool(name="t0p", bufs=4) as pt0,
            tc.tile_pool(name="lv1", bufs=3) as plv1,
            tc.tile_pool(name="lv2", bufs=2) as plv2,
            tc.tile_pool(name="t4p", bufs=1) as pt4,
            tc.tile_pool(name="tmp", bufs=2) as ptmp,
            tc.tile_pool(name="fin", bufs=1) as pfin,
        ):
            offs_t = pc.tile([128, 32], mybir.dt.int32, tag="offs",
                             name="offs_t")
            nc.gpsimd.dma_start(offs_t[:], offs_d[:])
            coef_t = pc.tile([128, _NCOLS], f32, tag="coef", name="coef_t")
            nc.sync.dma_start(coef_t[:], coef_d[:])
            warm_t = pc.tile([1, 8], f32, tag="warm", name="warm_t")
            nc.scalar.activation(warm_t[:], coef_t[0:1, 0:8],
                                 ACTF.Identity, bias=0.0, scale=1.0)

            def gather(t, split=False):
                at = pab.tile([128, FRUN], f16, tag="A", name=f"a{t}")
                bt = pab.tile([128, FRUN], f16, tag="B", name=f"b{t}")
                for side, dst in ((0, at), (1, bt)):
                    ioff = bass.IndirectOffsetOnAxis(
                        ap=offs_t[:, 2 * t + side:2 * t + side + 1], axis=0)
                    if split:
                        for (r0, r1) in HALVES:
                            nc.gpsimd.indirect_dma_start(
                                out=dst[:, 128 * r0:128 * r1],
                                out_offset=None, in_=xsrc_d[:],
                                in_offset=ioff, element_offset=128 * r0)
                    else:
                        nc.gpsimd.indirect_dma_start(
                            out=dst[:], out_offset=None, in_=xsrc_d[:],
                            in_offset=ioff)
                return at, bt

            gtiles = {0: gather(0, True), 1: gather(1, True)}
            tiles = {}

            def resolve_ab(i):
                """(a_ap, b_ap, base) for mix i; base = partition offset
                the u/w/q tmp lanes must live at (to match b for TT)."""
                op = _SCHED[i]
                l, key = op['level'], op['key']
                if l == 0:
                    at, bt = gtiles[key]
                    return at[:], bt[:], 0
                if l < 5:
                    return (tiles[(l - 1, 2 * key)][:],
                            tiles[(l - 1, 2 * key + 1)][:], 0)
                if l == 5:
                    t4 = tiles[(4, 0)]
                    return (t4[64 * key:64 * key + 32, :],
                            t4[64 * key + 32:64 * key + 64, :],
                            64 * key + 32)
                return tiles[(5, 0)][:], tiles[(5, 1)][:], 0

            def coefs(i, n):
                return [coef_t[0:n, 4 * i + j:4 * i + j + 1]
                        for j in range(4)]

            def w_on_sce(i):
                op = _SCHED[i]
                return (op['drain'] and op['level'] == 4) or \
                    (not op['drain'] and not op['hsplit']
                     and i % W_SCE_EVERY == W_SCE_EVERY - 1)

            def emit_u(i):
                """Pre-emit mix i's u-op (and its w-op when it rides ScalarE)
                — the lookahead keeps ScE a mix ahead of DVE."""
                op = _SCHED[i]
                n = op['lanes']
                a_ap, _, base = resolve_ab(i)
                c3, c2, c1, c0 = coefs(i, n)
                u_t = ptmp.tile([128, FRUN], f16, tag="u", name=f"u{i}")
                u_ap = u_t[base:base + n, :]
                chunks = HALVES if op['hsplit'] else FULL
                for (r0, r1) in chunks:
                    uv, av = vvr(u_ap, r0, r1), vvr(a_ap, r0, r1)
                    if op['drain'] and op['level'] >= 4:
                        nc.vector.tensor_scalar(uv, av, c3, c2,
                                                AL.mult, AL.add)
                    else:
                        nc.scalar.activation(uv, av, ACTF.Identity,
                                             bias=c2, scale=c3)
                return u_t, u_ap, None

            def can_lookahead(i):
                if i + 1 >= _NMIX:
                    return False
                nxt = _SCHED[i + 1]
                if nxt['level'] == 0:
                    return True
                a_child = ((5, 0) if nxt['level'] == 6 else
                           (4, 0) if nxt['level'] == 5 else
                           (nxt['level'] - 1, 2 * nxt['key']))
                cur = _SCHED[i]
                return a_child != (cur['level'], cur['key'])

            pend_accum = []
            pending_u = None
            for i, op in enumerate(_SCHED):
                l, key, n = op['level'], op['key'], op['lanes']
                _, _, c1, c0 = coefs(i, n)

                if l == 0 and key + 2 < 16:
                    gtiles[key + 2] = gather(key + 2)
                if pend_accum:
                    rp, up = pend_accum.pop(0)
                    nc.gpsimd.dma_start(out=r3d(rp[:]), in_=r3d(up[:]),
                                        accum_op=AL.add)

                a_ap, b_ap, base = resolve_ab(i)
                if pending_u is not None and pending_u[0] == i:
                    u_t, u_ap, w_ahead = pending_u[1]
                else:
                    u_t, u_ap, w_ahead = emit_u(i)
                pending_u = None

                # output tile
                if l == 0:
                    r_t = pt0.tile([128, FRUN], f16, tag="T0",
                                   name=f"t0_{key}")
                elif l == 1:
                    r_t = plv1.tile([128, FRUN], f16, tag="T1",
                                    name=f"t1_{key}")
                elif l < 4:
                    r_t = plv2.tile([128, FRUN], f16, tag=f"T{l}",
                                    name=f"t{l}_{key}")
                elif l == 4:
                    r_t = pt4.tile([128, FRUN], f16, tag="T4", name="t4")
                else:
                    r_t = pfin.tile([n, FRUN], f16, tag=f"T{l}_{key}",
                                    name=f"t{l}_{key}")
                tiles[(l, key)] = r_t

                accum = (ACCUM_L0 and l == 0 and key % 2 == 0
                         and not op['hsplit'])
                if accum:
                    w_ap = r_t[:]
                elif w_ahead is not None:
                    w_ap = w_ahead
                else:
                    w_t = ptmp.tile([128, FRUN], f16, tag="w", name=f"w{i}")
                    w_ap = w_t[base:base + n, :]

                chunks = HALVES if op['hsplit'] else FULL
                for (r0, r1) in chunks:
                    av, bv = vvr(a_ap, r0, r1), vvr(b_ap, r0, r1)
                    uv, wv = vvr(u_ap, r0, r1), vvr(w_ap, r0, r1)
                    rv = vvr(r_t[:], r0, r1)
                    if w_on_sce(i):
                        nc.scalar.activation(wv, av, ACTF.Identity,
                                             bias=c0, scale=c1)
                    else:
                        nc.vector.tensor_scalar(wv, av, c1, c0,
                                                AL.mult, AL.add)
                    nc.vector.tensor_tensor(uv, uv, bv, AL.mult)
                    if (r0, r1) == chunks[-1] and can_lookahead(i):
                        pending_u = (i + 1, emit_u(i + 1))
                    if not accum:
                        nc.vector.tensor_tensor(rv, uv, wv, AL.add)
                        if l == 6:
                            sl = slice(128 * r0, 128 * r1)
                            nc.sync.dma_start(out_d[:, sl], r_t[:, sl])
                if accum:
                    pend_accum.append((r_t, u_t))
    nc.compile()
    _BASS_CACHE['nc'] = nc
    return nc


def _prep_inputs(x, idx_a, idx_b, ws):
    x = np.ascontiguousarray(x, dtype=np.float32)
    in_maps = []
    for core in range(NCORES):
        g, h = core % 4, core // 4
        coef = _coef_tables(ws, h)
        offs = _offset_tables(idx_a, idx_b, h)
        # b4-interleaved slice: [C,H,W,B4] fp16
        xs = x[B4 * g:B4 * g + B4].transpose(1, 2, 3, 0)
        xp = np.zeros((XPAD,), dtype=np.float16)
        xp[:B4 * C * H * W] = xs.reshape(-1).astype(np.float16)
        in_maps.append({"xsrc": xp.reshape(XPAD, 1), "offs": offs,
                        "coef": coef})
    return in_maps


def _assemble(core_outs):
    """core_outs: list of [32, FRUN] f16 -> [16,64,900,1] f32."""
    full = np.empty((B, K, P, 1), dtype=np.float32)
    for core, o in enumerate(core_outs):
        g, h = core % 4, core // 4
        v = np.asarray(o).reshape(KH, PW, 32, B4)[:, :, 0:PW, :]  # k,hh,ww,b
        v = v.astype(np.float32).transpose(3, 0, 1, 2)            # b,k,hh,ww
        full[B4 * g:B4 * g + B4, KH * h:KH * h + KH] = \
            v.reshape(B4, KH, P, 1)
    return np.ascontiguousarray(full)


def kernel(x, idx_a, idx_b, w0, w1, w2, w3, w4, w5, w6):
    ws = [np.asarray(w, dtype=np.float32) for w in
          (w0, w1, w2, w3, w4, w5, w6)]
    x = np.asarray(x, dtype=np.float32)
    idx_a = np.asarray(idx_a, dtype=np.int32)
    idx_b = np.asarray(idx_b, dtype=np.int32)
    in_maps = _prep_inputs(x, idx_a, idx_b, ws)
    nc = _build_bass()
    from concourse.bass_utils import run_bass_kernel_spmd
    res = run_bass_kernel_spmd(nc, in_maps, core_ids=list(range(NCORES)))
    return _assemble([r["out"] for r in res.results])


def kernel_emulate(x, idx_a, idx_b, w0, w1, w2, w3, w4, w5, w6):
    """Pure-numpy emulation of the exact device schedule (debug aid)."""
    ws = [np.asarray(w, dtype=np.float32) for w in
          (w0, w1, w2, w3, w4, w5, w6)]
    in_maps = _prep_inputs(np.asarray(x, np.float32),
                           np.asarray(idx_a, np.int32),
                           np.asarray(idx_b, np.int32), ws)
    outs = [_emulate_core(m["xsrc"].reshape(-1), m["offs"], m["coef"])
            for m in in_maps]
    return _assemble(outs)
